# revision 1
# baseline (speedup 1.0000x reference)
"""Trainium2 Bass kernel for nn_ChebNet (complex Chebyshev GNN layer).

Sharding: data-parallel over batch B=8 across the 8 NeuronCores (one batch
element per core; attention/Chebyshev math is batch-independent, weights
replicated). No inter-core communication.

Per-core pipeline (batch b, N=1024 nodes, C=OC=64, K+1=5):
  A) Load X, build XT=[X_real^T; X_imag^T] (PE transposes), compute the 4
     attention projections with one packed matmul, broadcast dst rows.
  B) Per 128-row chunk of the attention matrix: s = prelu(src_i + dst_j),
     mag = sqrt(s_r^2+s_i^2), e = exp(mag), er = e/mag; PE ones-matmul
     accumulates the softmax column sums (softmax is over i = partitions).
  C) invc = 1/colsum, broadcast to a [128,1024] tile via rank-1 PE matmul.
  D) Per chunk: a = er*invc*s; stream L planes from HBM and compute
     SL[k,i] = sum_j L*a via fused multiply+row-reduce (DVE
     scalar_tensor_tensor with accum_out), 4 products per (k, chunk);
     then, riding the idle PE/ACT in the same chunk, the output einsum:
     A_k = X_real@w_r[k], B_k = X_imag@w_i[k] (PE), ACT scale-copies by
     the per-partition SL columns into a stacked tile, one DVE reduce
     per output -> out[n, o] chunks, DMA'd out at the end.

Perf notes (HW, NTFF profile): ~373 us/core. DVE is the roofline
(~343 us busy, 91%): the 4 fused multiply+rowsum products over
[5,1024,1024]x2 streams are irreducibly 160 DVE passes (fp32
tensor-tensor runs at 1 elem/lane/cycle; GPSIMD/walrus rejects
elementwise TT on Pool, PE fp32 is 4x derated so a diag-extraction
matmul scheme loses). HBM traffic 43.5 MB/core (~121 us at 358 GB/s)
fully hidden behind compute.
"""

import numpy as np
from contextlib import ExitStack

B, N, C, OC = 8, 1024, 64, 64
K1 = 5          # K+1 Chebyshev planes
P = 128         # partitions
CH = N // P     # 8 row chunks
NCORES = 8
EPS = 1e-30

_CACHE = {}


def _build_nc():
    import concourse.tile as tile
    from concourse import bacc, mybir

    f32 = mybir.dt.float32
    Alu = mybir.AluOpType
    Act = mybir.ActivationFunctionType

    nc = bacc.Bacc("TRN2", target_bir_lowering=False, debug=False,
                   enable_asserts=False, num_devices=NCORES)

    x_real = nc.dram_tensor("x_real", [N, C], f32, kind="ExternalInput").ap()
    x_imag = nc.dram_tensor("x_imag", [N, C], f32, kind="ExternalInput").ap()
    l_real = nc.dram_tensor("l_real", [K1, N, N], f32, kind="ExternalInput").ap()
    l_imag = nc.dram_tensor("l_imag", [K1, N, N], f32, kind="ExternalInput").ap()
    w4 = nc.dram_tensor("w4", [2 * C, 4], f32, kind="ExternalInput").ap()
    wcat_r = nc.dram_tensor("wcat_r", [2 * C, K1 * OC], f32, kind="ExternalInput").ap()
    wcat_i = nc.dram_tensor("wcat_i", [2 * C, K1 * OC], f32, kind="ExternalInput").ap()
    pa_cols = nc.dram_tensor("pa_cols", [P, 2], f32, kind="ExternalInput").ap()
    ab2 = nc.dram_tensor("ab2", [2, 1], f32, kind="ExternalInput").ap()
    out_r = nc.dram_tensor("out_r", [N, OC], f32, kind="ExternalOutput").ap()
    out_i = nc.dram_tensor("out_i", [N, OC], f32, kind="ExternalOutput").ap()

    with tile.TileContext(nc) as tc:
        with ExitStack() as ctx:
            _emit(ctx, tc, nc, mybir, f32, Alu, Act,
                  x_real, x_imag, l_real, l_imag, w4, wcat_r, wcat_i,
                  pa_cols, ab2, out_r, out_i)
    nc.compile()
    return nc


def _emit(ctx, tc, nc, mybir, f32, Alu, Act,
          x_real, x_imag, l_real, l_imag, w4, wcat_r, wcat_i,
          pa_cols, ab2, out_r, out_i):
    # ---------------- persistent pools / constants ----------------
    const = ctx.enter_context(tc.tile_pool(name="const", bufs=1))
    persist = ctx.enter_context(tc.tile_pool(name="persist", bufs=1))
    tp_psum = ctx.enter_context(tc.tile_pool(name="tp_psum", bufs=2, space="PSUM"))

    ident_i = const.tile([P, P], mybir.dt.int32)
    nc.gpsimd.iota(ident_i[:], pattern=[[1, P]], base=0, channel_multiplier=-1)
    ident = const.tile([P, P], f32)
    nc.vector.tensor_scalar(ident[:], ident_i[:], 0, None, op0=Alu.is_equal)

    ones_row = const.tile([1, P], f32)
    nc.vector.memset(ones_row[:], 1.0)
    ones_col = const.tile([P, 1], f32)
    nc.vector.memset(ones_col[:], 1.0)
    eps_col = const.tile([P, 1], f32)
    nc.vector.memset(eps_col[:], EPS)
    zero_col = const.tile([P, 1], f32)
    nc.vector.memset(zero_col[:], 0.0)

    w4_sb = const.tile([P, 4], f32)
    nc.sync.dma_start(w4_sb[:], w4[:])
    wcr_sb = const.tile([P, K1 * OC], f32)
    nc.sync.dma_start(wcr_sb[:], wcat_r[:])
    wci_sb = const.tile([P, K1 * OC], f32)
    nc.sync.dma_start(wci_sb[:], wcat_i[:])
    pa_sb = const.tile([P, 2], f32)
    nc.sync.dma_start(pa_sb[:], pa_cols[:])
    ab_sb = const.tile([2, 1], f32)
    nc.sync.dma_start(ab_sb[:], ab2[:])

    xt = persist.tile([P, N], f32)          # [ [X_real^T ; X_imag^T] , n ]
    srcT = persist.tile([P, 2 * CH], f32)   # per chunk: col 2c = src_r, 2c+1 = src_i
    dst_bc_r = persist.tile([P, N], f32)
    dst_bc_i = persist.tile([P, N], f32)
    invc_b = persist.tile([P, N], f32)

    sl_t = []
    slp = ctx.enter_context(tc.tile_pool(name="slp", bufs=CH))
    for c in range(CH):
        sl_t.append(slp.tile([P, 2 * K1], f32, tag="slp", name=f"sl{c}"))

    # ---------------- phase A: XT, projections, dst broadcast ----------------
    with tc.tile_pool(name="phaseA", bufs=1) as pa_pool, \
         tc.tile_pool(name="pa_psum", bufs=2, space="PSUM") as pa_psum:
        xr_sb = pa_pool.tile([P, CH, C], f32)
        xi_sb = pa_pool.tile([P, CH, C], f32)
        nc.sync.dma_start(xr_sb[:], x_real.rearrange("(c p) d -> p c d", p=P))
        nc.sync.dma_start(xi_sb[:], x_imag.rearrange("(c p) d -> p c d", p=P))

        for c in range(CH):
            tp = tp_psum.tile([C, P], f32, tag="tp")
            nc.tensor.transpose(tp[:], xr_sb[:, c, :], ident[:])
            nc.vector.tensor_copy(xt[0:C, c * P:(c + 1) * P], tp[:])
            tp2 = tp_psum.tile([C, P], f32, tag="tp")
            nc.tensor.transpose(tp2[:], xi_sb[:, c, :], ident[:])
            nc.scalar.copy(xt[C:2 * C, c * P:(c + 1) * P], tp2[:])

        # projections (separate tiles so each starts at partition 0):
        # src [2, n] = (ws_r|ws_i)^T @ XT ; dst_r / dst_i [1, n]
        src_ps = pa_psum.tile([2, N], f32, tag="proj")
        dstr_ps = pa_psum.tile([1, N], f32, tag="proj")
        dsti_ps = pa_psum.tile([1, N], f32, tag="proj")
        for h in range(2):
            hs = slice(h * 512, (h + 1) * 512)
            nc.tensor.matmul(src_ps[:, hs], w4_sb[:, 0:2], xt[:, hs],
                             start=True, stop=True)
            nc.tensor.matmul(dstr_ps[:, hs], w4_sb[:, 2:3], xt[:, hs],
                             start=True, stop=True)
            nc.tensor.matmul(dsti_ps[:, hs], w4_sb[:, 3:4], xt[:, hs],
                             start=True, stop=True)
        src_sb = pa_pool.tile([2, N], f32)
        nc.scalar.copy(src_sb[:], src_ps[:])
        dstr_sb = pa_pool.tile([1, N], f32)
        nc.scalar.copy(dstr_sb[:], dstr_ps[:])
        dsti_sb = pa_pool.tile([1, N], f32)
        nc.scalar.copy(dsti_sb[:], dsti_ps[:])
        # += attention bias (ab_real to src_r row, ab_imag to src_i row)
        nc.vector.tensor_scalar(src_sb[:], src_sb[:],
                                ab_sb[:, 0:1], None, op0=Alu.add)

        # srcT: per-chunk transpose of src rows -> [p, 2] columns
        for c in range(CH):
            tp3 = tp_psum.tile([P, 2], f32, tag="tp")
            nc.tensor.transpose(tp3[:], src_sb[:, c * P:(c + 1) * P],
                                ident[0:2, 0:2])
            nc.vector.tensor_copy(srcT[:, 2 * c:2 * c + 2], tp3[:])

        # dst broadcast tiles via rank-1 matmuls
        for row_sb, dstt in ((dstr_sb, dst_bc_r), (dsti_sb, dst_bc_i)):
            for h in range(2):
                bc = pa_psum.tile([P, 512], f32, tag="dstbc")
                nc.tensor.matmul(bc[:], ones_row[:],
                                 row_sb[:, h * 512:(h + 1) * 512],
                                 start=True, stop=True)
                if h == 0:
                    nc.vector.tensor_copy(dstt[:, 0:512], bc[:])
                else:
                    nc.scalar.copy(dstt[:, 512:1024], bc[:])

    # ---------------- phases B-D share the big s/er store ----------------
    sp_stack = ExitStack()
    sp = sp_stack.enter_context(tc.tile_pool(name="spers", bufs=3 * CH))
    sr_t, si_t, er_t = [], [], []
    for c in range(CH):
        sr_t.append(sp.tile([P, N], f32, tag="spers", name=f"sr{c}"))
        si_t.append(sp.tile([P, N], f32, tag="spers", name=f"si{c}"))
        er_t.append(sp.tile([P, N], f32, tag="spers", name=f"er{c}"))

    # ---------------- phase B: attention pass 1 + softmax colsums ----------------
    with tc.tile_pool(name="phaseB", bufs=2) as pb_pool, \
         tc.tile_pool(name="cs_psum", bufs=1, space="PSUM") as cs_psum:
        colsum = cs_psum.tile([1, N], f32)
        for c in range(CH):
            sr = sr_t[c]
            si = si_t[c]
            # s = dst_bc + src (+ab already folded into src rows)
            nc.scalar.activation(sr[:], dst_bc_r[:], Act.Identity,
                                 bias=srcT[:, 2 * c:2 * c + 1], scale=1.0)
            nc.scalar.activation(si[:], dst_bc_i[:], Act.Identity,
                                 bias=srcT[:, 2 * c + 1:2 * c + 2], scale=1.0)
            # CPReLU: max(pa*s, s), in place (stt is DVE-only)
            nc.vector.scalar_tensor_tensor(sr[:], sr[:], pa_sb[:, 0:1], sr[:],
                                           op0=Alu.mult, op1=Alu.max)
            nc.vector.scalar_tensor_tensor(si[:], si[:], pa_sb[:, 1:2], si[:],
                                           op0=Alu.mult, op1=Alu.max)
            # mag^2 = sr^2 + si^2
            sqr = pb_pool.tile([P, N], f32, tag="sqr", bufs=3)
            nc.scalar.activation(sqr[:], sr[:], Act.Square, bias=zero_col[:, 0:1])
            m2 = pb_pool.tile([P, N], f32, tag="m2", bufs=3)
            nc.scalar.activation(m2[:], si[:], Act.Square, bias=zero_col[:, 0:1])
            nc.vector.tensor_add(m2[:], m2[:], sqr[:])
            # mag = sqrt(m2 + eps); e = exp(mag); er = e / mag
            mag = pb_pool.tile([P, N], f32, tag="mag", bufs=3)
            nc.scalar.activation(mag[:], m2[:], Act.Sqrt, bias=eps_col[:, 0:1])
            e = pb_pool.tile([P, N], f32, tag="e", bufs=3)
            nc.scalar.activation(e[:], mag[:], Act.Exp, bias=zero_col[:, 0:1])
            rmag = pb_pool.tile([P, N], f32, tag="rmag")
            nc.vector.reciprocal_approx_fast(rmag[:], mag[:])
            nc.vector.tensor_mul(er_t[c][:], e[:], rmag[:])
            # softmax column sums (over i = partitions) via PE
            nc.tensor.matmul(colsum[:, 0:512], ones_col[:], e[:, 0:512],
                             start=(c == 0), stop=(c == CH - 1))
            nc.tensor.matmul(colsum[:, 512:1024], ones_col[:], e[:, 512:1024],
                             start=(c == 0), stop=(c == CH - 1))

        # ---------------- phase C: 1/colsum broadcast ----------------
        with tc.tile_pool(name="inv_psum", bufs=1, space="PSUM") as inv_psum:
            invc = pb_pool.tile([1, N], f32, tag="invc", bufs=1)
            cs_sb = pb_pool.tile([1, N], f32, tag="cssb", bufs=1)
            nc.vector.tensor_copy(cs_sb[:], colsum[:])
            ivscr = pb_pool.tile([1, N], f32, tag="ivscr", bufs=1)
            nc.vector.reciprocal_approx_accurate(invc[:], cs_sb[:], ivscr[:])
            for h in range(2):
                ib = inv_psum.tile([P, 512], f32, tag="invbc")
                nc.tensor.matmul(ib[:], ones_row[:],
                                 invc[:, h * 512:(h + 1) * 512],
                                 start=True, stop=True)
                nc.scalar.copy(invc_b[:, h * 512:(h + 1) * 512], ib[:])

    # ---------------- phase D: modulation + row sums ----------------
    # DVE handles k in 0..2 via chained tensor_tensor_reduce;
    # GPSIMD handles k in 3..4 via stt+accum into pacc, combined on DVE.
    with tc.tile_pool(name="lr_pool", bufs=5) as lrp, \
         tc.tile_pool(name="li_pool", bufs=5) as lip, \
         tc.tile_pool(name="dmod", bufs=2) as dmod, \
         tc.tile_pool(name="scr_pool", bufs=2) as scrp, \
         tc.tile_pool(name="stack_pool", bufs=1) as stackp, \
         tc.tile_pool(name="ab_psum", bufs=2, space="PSUM") as abps, \
         tc.tile_pool(name="pacc_pool", bufs=2) as paccp:
        or_sb = persist.tile([P, CH * OC], f32)
        oi_sb = persist.tile([P, CH * OC], f32)
        for c in range(CH):
            g = dmod.tile([P, N], f32, tag="g")
            nc.vector.tensor_mul(g[:], er_t[c][:], invc_b[:])
            ar = dmod.tile([P, N], f32, tag="ar", bufs=3)
            nc.vector.tensor_mul(ar[:], g[:], sr_t[c][:])
            ai = dmod.tile([P, N], f32, tag="ai", bufs=3)
            nc.vector.tensor_mul(ai[:], g[:], si_t[c][:])

            sl = sl_t[c]
            # stt+accum_out: slot q of pacc gets sum_j of the q-th product;
            # slots per k: [Lr*ar, -Li*ai, Lr*ai, Li*ar]
            pacc = paccp.tile([P, 4 * K1], f32, tag="pacc")
            for k in range(K1):
                lr = lrp.tile([P, N], f32, tag="lr")
                nc.sync.dma_start(lr[:], l_real[k, c * P:(c + 1) * P, :])
                li = lip.tile([P, N], f32, tag="li")
                nc.sync.dma_start(li[:], l_imag[k, c * P:(c + 1) * P, :])
                for idx, (ta, tb, sc) in enumerate(((lr, ar, 1.0),
                                                    (li, ai, -1.0),
                                                    (lr, ai, 1.0),
                                                    (li, ar, 1.0))):
                    sg = scrp.tile([P, N], f32, tag="scr")
                    op0 = Alu.bypass if sc == 1.0 else Alu.mult
                    nc.vector.scalar_tensor_tensor(
                        sg[:], ta[:], sc, tb[:], op0=op0, op1=Alu.mult,
                        accum_out=pacc[:, 4 * k + idx:4 * k + idx + 1])
            # SLr[k] = slot0 + slot1 ; SLi[k] = slot2 + slot3  (strided views)
            p4 = pacc[:].rearrange("p (k i) -> p k i", i=4)
            nc.vector.tensor_add(
                sl[:, 0:K1].rearrange("p (k i) -> p k i", i=1),
                p4[:, :, 0:1], p4[:, :, 1:2])
            nc.vector.tensor_add(
                sl[:, K1:2 * K1].rearrange("p (k i) -> p k i", i=1),
                p4[:, :, 2:3], p4[:, :, 3:4])
            # mSLi = -SLi for the out_r accumulation
            msli = paccp.tile([P, K1], f32, tag="msli")
            nc.vector.tensor_scalar(msli[:], sl[:, K1:2 * K1], -1.0, None,
                                    op0=Alu.mult)

            # ---- output einsum for this chunk (rides idle PE + ACT) ----
            # A_k = X_real @ w_r[k], B_k = X_imag @ w_i[k]  ([128, 64] PSUM)
            # out_r = sum_k slr_k*A_k - sli_k*B_k ; out_i = sli_k*A_k + slr_k*B_k
            stk_r = stackp.tile([P, OC, 2 * K1], f32, tag="stkr")
            stk_i = stackp.tile([P, OC, 2 * K1], f32, tag="stki")
            for k in range(K1):
                ks = slice(k * OC, (k + 1) * OC)
                apk = abps.tile([P, OC], f32, tag="apk")
                nc.tensor.matmul(apk[:], xt[0:C, c * P:(c + 1) * P],
                                 wci_sb[0:C, ks], start=True, stop=True)
                bpk = abps.tile([P, OC], f32, tag="bpk")
                nc.tensor.matmul(bpk[:], xt[C:2 * C, c * P:(c + 1) * P],
                                 wci_sb[C:2 * C, ks], start=True, stop=True)
                nc.scalar.activation(stk_r[:, :, 2 * k:2 * k + 1], apk[:],
                                     Act.Copy, scale=sl[:, k:k + 1])
                nc.scalar.activation(stk_r[:, :, 2 * k + 1:2 * k + 2], bpk[:],
                                     Act.Copy, scale=msli[:, k:k + 1])
                nc.scalar.activation(stk_i[:, :, 2 * k:2 * k + 1], apk[:],
                                     Act.Copy, scale=sl[:, K1 + k:K1 + k + 1])
                nc.scalar.activation(stk_i[:, :, 2 * k + 1:2 * k + 2], bpk[:],
                                     Act.Copy, scale=sl[:, k:k + 1])
            os_ = slice(c * OC, (c + 1) * OC)
            nc.vector.reduce_sum(or_sb[:, os_], stk_r[:],
                                 axis=mybir.AxisListType.X)
            nc.vector.reduce_sum(oi_sb[:, os_], stk_i[:],
                                 axis=mybir.AxisListType.X)
        nc.sync.dma_start(out_r.rearrange("(c p) o -> p c o", p=P),
                          or_sb[:].rearrange("p (c o) -> p c o", c=CH))
        nc.sync.dma_start(out_i.rearrange("(c p) o -> p c o", p=P),
                          oi_sb[:].rearrange("p (c o) -> p c o", c=CH))

    sp_stack.close()

def _host_prep(inputs):
    """Build per-core input maps from full inputs."""
    f = lambda k: np.ascontiguousarray(np.asarray(inputs[k], dtype=np.float32))
    X_real, X_imag = f("X_real"), f("X_imag")
    L_real, L_imag = f("L_real"), f("L_imag")
    w_real, w_imag = f("w_real"), f("w_imag")
    aw_real, aw_imag = f("aw_real"), f("aw_imag")
    ab_real = float(np.asarray(inputs["ab_real"]))
    ab_imag = float(np.asarray(inputs["ab_imag"]))
    pa_real = float(np.asarray(inputs["pa_real"]))
    pa_imag = float(np.asarray(inputs["pa_imag"]))

    ws_r, wd_r = aw_real[:C], aw_real[C:]
    ws_i, wd_i = aw_imag[:C], aw_imag[C:]
    w4 = np.stack([
        np.concatenate([ws_r, -ws_i]),
        np.concatenate([ws_i, ws_r]),
        np.concatenate([wd_r, -wd_i]),
        np.concatenate([wd_i, wd_r]),
    ], axis=1).astype(np.float32)                      # [128, 4]

    # wcat_r = [w_r[k]; -w_i[k]] over c2, free=(k,o); wcat_i = [w_r[k]; w_i[k]]
    wr_t = w_real.transpose(1, 0, 2).reshape(C, K1 * OC)
    wi_t = w_imag.transpose(1, 0, 2).reshape(C, K1 * OC)
    wcat_r = np.concatenate([wr_t, -wi_t], axis=0).astype(np.float32)
    wcat_i = np.concatenate([wr_t, wi_t], axis=0).astype(np.float32)

    pa_cols = np.stack([np.full(P, pa_real), np.full(P, pa_imag)],
                       axis=1).astype(np.float32)       # [128, 2]
    ab2 = np.array([[ab_real], [ab_imag]], dtype=np.float32)

    in_maps = []
    for b in range(NCORES):
        in_maps.append({
            "x_real": X_real[b], "x_imag": X_imag[b],
            "l_real": L_real[b], "l_imag": L_imag[b],
            "w4": w4, "wcat_r": wcat_r, "wcat_i": wcat_i,
            "pa_cols": pa_cols, "ab2": ab2,
        })
    return in_maps


def kernel(**inputs):
    import os
    from concourse import bass_utils

    if "nc" not in _CACHE:
        _CACHE["nc"] = _build_nc()
    nc = _CACHE["nc"]
    in_maps = _host_prep(inputs)
    trace = os.environ.get("KERNEL_TRACE", "0") == "1"
    res = bass_utils.run_bass_kernel_spmd(nc, in_maps,
                                          core_ids=list(range(NCORES)),
                                          trace=trace)
    _CACHE["last_result"] = res
    out_r = np.stack([res.results[b]["out_r"] for b in range(NCORES)])
    out_i = np.stack([res.results[b]["out_i"] for b in range(NCORES)])
    return out_r, out_i



# revision 31
# speedup vs baseline: 1.7492x; 1.7492x over previous
"""Trainium2 Bass kernel for nn_ChebNet (complex Chebyshev GNN layer).

Sharding: data-parallel over batch B=8 across the 8 NeuronCores (one batch
element per core; weights replicated). No inter-core communication.

v2 design — transposed domain. All [N,N] work uses tiles [j-part, i-free]
(the baseline used [i-part, j-free]). Wins:
  * softmax (over i) denominator = free-axis row-sum -> rides the ACT Exp
    pass via accum_out; 1/colsum[j] and 1/mag become per-partition exp-bias
    folds (no broadcast machinery, no DVE reciprocal).
  * products L^T (.) a^T are plain bf16 tensor_tensor at DVE 2x_1P mode
    (690ns vs the baseline's scalar_tensor_tensor which has NO perf modes,
    1226ns), with the j-reduction done on the idle PE via +/-1 pair-column
    ones-matmuls accumulating SLr/SLi rows straight in PSUM.
  * Gauss 3-mult complex trick: P1=Lr.ar, P2=Li.ai, P3=(Lr+Li).(ar+ai);
    SLr=P1-P2, SLi=P3-P1-P2 -> 15 instead of 20 product passes per chunk,
    at the cost of a third (host-precomputed) L-stream.
  * L streams sent as bf16 (30MB vs 40MB fp32) on 3 separate engine DMA
    queues (sync/gpsimd/tensor) to break the single-queue 250GB/s limit.
  * output einsum out^T = sum_k wcat_k^T @ (X^T (.) SL_k-bcast): all-PE
    bf16 matmuls; SL row-broadcasts on the idle GPSIMD. Output leaves the
    device transposed [OC, N]; the host untransposes (layout only).

PReLU fuses into the s-construction ACT op (func=Prelu, bias=dst column,
alpha=pa per-partition column). exp/ln/square/prelu/copy all live in the
single `natural_log_exp_and_others` ACT table set -> one table load total
(the baseline paid 17 loads from Sqrt/Exp set thrash).

Precision: attention chain (s, mag, exp) fp32 end-to-end; only the
modulation weights ar/ai, the L tiles, and the output einsum are bf16.
"""

import numpy as np
from contextlib import ExitStack

B, N, C, OC = 8, 1024, 64, 64
K1 = 5          # K+1 Chebyshev planes
P = 128         # partitions
CH = N // P     # 8 j-chunks
NCORES = 8
EPS = 1e-30

_CACHE = {}
DEBUG = False   # add intermediate-dump outputs (debugging only)


def _build_nc():
    import concourse.tile as tile
    from concourse import bacc, mybir

    f32 = mybir.dt.float32
    bf16 = mybir.dt.bfloat16
    Alu = mybir.AluOpType
    Act = mybir.ActivationFunctionType

    nc = bacc.Bacc("TRN2", target_bir_lowering=False, debug=False,
                   enable_asserts=False, num_devices=NCORES)

    x_real = nc.dram_tensor("x_real", [N, C], f32, kind="ExternalInput").ap()
    x_imag = nc.dram_tensor("x_imag", [N, C], f32, kind="ExternalInput").ap()
    lrt = nc.dram_tensor("lrt", [CH, K1, P, N], bf16, kind="ExternalInput").ap()
    lit = nc.dram_tensor("lit", [CH, K1, P, N], bf16, kind="ExternalInput").ap()
    lst = nc.dram_tensor("lst", [CH, K1, P, N], bf16, kind="ExternalInput").ap()
    w4 = nc.dram_tensor("w4", [2 * C, 4], f32, kind="ExternalInput").ap()
    wcat_r = nc.dram_tensor("wcat_r", [2 * C, K1 * OC], bf16, kind="ExternalInput").ap()
    wcat_i = nc.dram_tensor("wcat_i", [2 * C, K1 * OC], bf16, kind="ExternalInput").ap()
    pa_cols = nc.dram_tensor("pa_cols", [P, 2], f32, kind="ExternalInput").ap()
    abr = nc.dram_tensor("abr", [1, 1], f32, kind="ExternalInput").ap()
    abi = nc.dram_tensor("abi", [1, 1], f32, kind="ExternalInput").ap()
    out_rt = nc.dram_tensor("out_rt", [OC, N], f32, kind="ExternalOutput").ap()
    out_it = nc.dram_tensor("out_it", [OC, N], f32, kind="ExternalOutput").ap()
    sl_dram = nc.dram_tensor("sl_dram", [2 * K1, N], bf16, kind="Internal").ap()
    dbg = None
    if DEBUG:
        dbg = {
            "sr0": nc.dram_tensor("d_sr0", [P, N], f32, kind="ExternalOutput").ap(),
            "t0": nc.dram_tensor("d_t0", [P, N], f32, kind="ExternalOutput").ap(),
            "cs": nc.dram_tensor("d_cs", [P, CH], f32, kind="ExternalOutput").ap(),
            "ar0": nc.dram_tensor("d_ar0", [P, N], bf16, kind="ExternalOutput").ap(),
            "p10": nc.dram_tensor("d_p10", [P, N], bf16, kind="ExternalOutput").ap(),
            "slbf": nc.dram_tensor("d_slbf", [2 * K1, N], bf16, kind="ExternalOutput").ap(),
            "b120": nc.dram_tensor("d_b120", [P, N], bf16, kind="ExternalOutput").ap(),
            "v120": nc.dram_tensor("d_v120", [P, N], bf16, kind="ExternalOutput").ap(),
        }

    with tile.TileContext(nc) as tc:
        with ExitStack() as ctx:
            _emit(ctx, tc, nc, mybir, f32, bf16, Alu, Act,
                  x_real, x_imag, lrt, lit, lst, w4, wcat_r, wcat_i,
                  pa_cols, abr, abi, out_rt, out_it, sl_dram, dbg)
    nc.compile()
    return nc


def _emit(ctx, tc, nc, mybir, f32, bf16, Alu, Act,
          x_real, x_imag, lrt, lit, lst, w4, wcat_r, wcat_i,
          pa_cols, abr, abi, out_rt, out_it, sl_dram=None, dbg=None):

    def ddump(name, tile_ap, c=None):
        if dbg is None or name not in dbg:
            return
        if c is None:
            nc.sync.dma_start(dbg[name][:], tile_ap)
        else:
            nc.sync.dma_start(dbg[name][:, c:c + 1], tile_ap)
    # ---------------- constants / persistent tiles ----------------
    const = ctx.enter_context(tc.tile_pool(name="const", bufs=1))
    persist = ctx.enter_context(tc.tile_pool(name="persist", bufs=1))

    ident_i = const.tile([P, P], mybir.dt.int32)
    nc.gpsimd.iota(ident_i[:], pattern=[[1, P]], base=0, channel_multiplier=-1)
    ident = const.tile([P, P], f32)
    nc.vector.tensor_scalar(ident[:], ident_i[:], 0, None, op0=Alu.is_equal)

    ones_row = const.tile([1, P], f32)
    nc.vector.memset(ones_row[:], 1.0)
    zero_col = const.tile([P, 1], f32)
    nc.vector.memset(zero_col[:], 0.0)
    eps_col = const.tile([P, 1], f32)
    nc.vector.memset(eps_col[:], EPS)

    # ones-reduction weights for the PE product reductions (bf16).
    # SL PSUM rows: 2k = SLr_k, 2k+1 = SLi_k. Per (k, product) a [P, 10]
    # lhsT with +/-1 only in columns (2k, 2k+1):
    #   P1 -> (+SLr, -SLi), P2 -> (-SLr, -SLi), P3 -> (0, +SLi)
    # (zero columns add 0 to the other k rows — accumulation-safe).
    PSIGNS = ((1.0, -1.0), (-1.0, -1.0), (0.0, 1.0))
    pairw = []
    for k in range(K1):
        row = []
        for (s0, s1) in PSIGNS:
            t = const.tile([P, 2 * K1], bf16, name=f"pw{k}_{len(row)}")
            nc.vector.memset(t[:], 0.0)
            if s0 != 0.0:
                nc.vector.memset(t[:, 2 * k:2 * k + 1], s0)
            if s1 != 0.0:
                nc.vector.memset(t[:, 2 * k + 1:2 * k + 2], s1)
            row.append(t)
        pairw.append(row)

    w4_sb = const.tile([P, 4], f32)
    nc.sync.dma_start(w4_sb[:], w4[:])
    wcr_sb = const.tile([P, K1 * OC], bf16)
    nc.sync.dma_start(wcr_sb[:], wcat_r[:])
    wci_sb = const.tile([P, K1 * OC], bf16)
    nc.sync.dma_start(wci_sb[:], wcat_i[:])
    pa_sb = const.tile([P, 2], f32)
    nc.sync.dma_start(pa_sb[:], pa_cols[:])
    abr_sb = const.tile([1, 1], f32)
    nc.sync.dma_start(abr_sb[:], abr[:])
    abi_sb = const.tile([1, 1], f32)
    nc.sync.dma_start(abi_sb[:], abi[:])

    xt = persist.tile([P, N], f32)            # [Xr^T ; Xi^T], free = i
    xt_bf = persist.tile([P, N], bf16)
    src_bc_r = persist.tile([P, N], f32)      # src broadcast over j-partitions
    src_bc_i = persist.tile([P, N], f32)
    dstT = persist.tile([P, 2 * CH], f32)     # per chunk: col 2c=dst_r, 2c+1=dst_i

    # ---------------- phase A: xt, projections, broadcasts ----------------
    with tc.tile_pool(name="phaseA", bufs=1) as pa_pool, \
         tc.tile_pool(name="pa_psum", bufs=1, space="PSUM") as pa_psum, \
         tc.tile_pool(name="tp_psum", bufs=1, space="PSUM") as tp_psum:
        xr_sb = pa_pool.tile([P, CH, C], f32)
        xi_sb = pa_pool.tile([P, CH, C], f32)
        nc.sync.dma_start(xr_sb[:], x_real.rearrange("(c p) d -> p c d", p=P))
        nc.sync.dma_start(xi_sb[:], x_imag.rearrange("(c p) d -> p c d", p=P))

        for c in range(CH):
            tp = tp_psum.tile([C, P], f32, tag="tp", bufs=2)
            nc.tensor.transpose(tp[:], xr_sb[:, c, :], ident[:])
            nc.vector.tensor_copy(xt[0:C, c * P:(c + 1) * P], tp[:])
            tp2 = tp_psum.tile([C, P], f32, tag="tp", bufs=2)
            nc.tensor.transpose(tp2[:], xi_sb[:, c, :], ident[:])
            nc.scalar.copy(xt[C:2 * C, c * P:(c + 1) * P], tp2[:])
        nc.vector.tensor_copy(xt_bf[:], xt[:])

        # projections: src_r / src_i [1, n] (separate tiles -> partition 0),
        # dst [2, n] (only consumed via transpose)
        srcr_ps = pa_psum.tile([1, N], f32, tag="proj", bufs=2)
        srci_ps = pa_psum.tile([1, N], f32, tag="proj", bufs=2)
        dst_ps = pa_psum.tile([2, N], f32, tag="proj", bufs=2)
        for h in range(2):
            hs = slice(h * 512, (h + 1) * 512)
            nc.tensor.matmul(srcr_ps[:, hs], w4_sb[:, 0:1], xt[:, hs],
                             start=True, stop=True)
            nc.tensor.matmul(srci_ps[:, hs], w4_sb[:, 1:2], xt[:, hs],
                             start=True, stop=True)
            nc.tensor.matmul(dst_ps[:, hs], w4_sb[:, 2:4], xt[:, hs],
                             start=True, stop=True)
        # copy to SBUF folding the attention bias ab into src rows
        srcr_sb = pa_pool.tile([1, N], f32)
        nc.scalar.activation(srcr_sb[:], srcr_ps[:], Act.Identity,
                             bias=abr_sb[:, 0:1], scale=1.0)
        srci_sb = pa_pool.tile([1, N], f32)
        nc.scalar.activation(srci_sb[:], srci_ps[:], Act.Identity,
                             bias=abi_sb[:, 0:1], scale=1.0)
        dst_sb = pa_pool.tile([2, N], f32)
        nc.scalar.copy(dst_sb[:], dst_ps[:])

        # src broadcast tiles (rank-1 PE) -> [128, 1024]
        for row_sb, dstt in ((srcr_sb, src_bc_r), (srci_sb, src_bc_i)):
            for h in range(2):
                bc = pa_psum.tile([P, 512], f32, tag="srcbc")
                nc.tensor.matmul(bc[:], ones_row[:],
                                 row_sb[:, h * 512:(h + 1) * 512],
                                 start=True, stop=True)
                nc.scalar.copy(dstt[:, h * 512:(h + 1) * 512], bc[:])

        # dstT: per-chunk transpose of dst rows -> [p, 2] columns
        for c in range(CH):
            tp3 = tp_psum.tile([P, 2], f32, tag="tpd")
            nc.tensor.transpose(tp3[:], dst_sb[:, c * P:(c + 1) * P],
                                ident[0:2, 0:2])
            nc.vector.tensor_copy(dstT[:, 2 * c:2 * c + 2], tp3[:])

    # ---------------- main loop: attention + products, per j-chunk ----------------
    # SL accumulates in PSUM rows: 2k = SLr_k, 2k+1 = SLi_k
    sl_psum_ctx = ExitStack()
    sl_pool = sl_psum_ctx.enter_context(
        tc.tile_pool(name="sl_psum", bufs=1, space="PSUM"))
    slp = sl_pool.tile([2 * K1, N], f32)

    with tc.tile_pool(name="sbig", bufs=2) as sbig, \
         tc.tile_pool(name="sbf", bufs=2) as sbf, \
         tc.tile_pool(name="lr_pool", bufs=7) as lrp, \
         tc.tile_pool(name="li_pool", bufs=7) as lip, \
         tc.tile_pool(name="ls_pool", bufs=7) as lsp, \
         tc.tile_pool(name="prod_pool", bufs=6) as prp, \
         tc.tile_pool(name="col_pool", bufs=2) as colp:
        for c in range(CH):
            # --- L prefetch for this chunk on 3 independent queues ---
            lr_t, li_t, ls_t = [], [], []
            for k in range(K1):
                lr = lrp.tile([P, N], bf16, tag="lr")
                nc.sync.dma_start(lr[:], lrt[c, k])
                lr_t.append(lr)
                li = lip.tile([P, N], bf16, tag="li")
                nc.gpsimd.dma_start(li[:], lit[c, k])
                li_t.append(li)
                ls = lsp.tile([P, N], bf16, tag="ls")
                nc.scalar.dma_start(ls[:], lst[c, k])
                ls_t.append(ls)

            # --- attention chain (ACT-heavy, fp32) ---
            sr = sbig.tile([P, N], f32, tag="sr")
            nc.scalar.activation(sr[:], src_bc_r[:], Act.Prelu,
                                 bias=dstT[:, 2 * c:2 * c + 1], scale=1.0,
                                 alpha=pa_sb[:, 0:1])
            si = sbig.tile([P, N], f32, tag="si")
            nc.scalar.activation(si[:], src_bc_i[:], Act.Prelu,
                                 bias=dstT[:, 2 * c + 1:2 * c + 2], scale=1.0,
                                 alpha=pa_sb[:, 1:2])
            sqr = sbig.tile([P, N], f32, tag="sqr")
            nc.scalar.activation(sqr[:], sr[:], Act.Square, bias=zero_col[:, 0:1])
            sqi = sbig.tile([P, N], f32, tag="sqi")
            nc.scalar.activation(sqi[:], si[:], Act.Square, bias=zero_col[:, 0:1])
            m2 = sbig.tile([P, N], f32, tag="m2")
            nc.vector.tensor_add(m2[:], sqr[:], sqi[:])
            u = sbig.tile([P, N], f32, tag="u")
            nc.scalar.activation(u[:], m2[:], Act.Ln, bias=eps_col[:, 0:1])
            mag = sbig.tile([P, N], f32, tag="mag")
            nc.scalar.activation(mag[:], u[:], Act.Exp, scale=0.5,
                                 bias=zero_col[:, 0:1])
            # e = exp(mag); accum_out = colsum (softmax denominator, axis=i)
            ejunk = sbf.tile([P, N], bf16, tag="ejunk")
            colsum = colp.tile([P, 1], f32, tag="colsum")
            nc.scalar.activation(ejunk[:], mag[:], Act.Exp,
                                 bias=zero_col[:, 0:1], accum_out=colsum[:])
            lnc = colp.tile([P, 1], f32, tag="lnc")
            nc.scalar.activation(lnc[:], colsum[:], Act.Ln, bias=zero_col[:, 0:1])
            lnv = colp.tile([P, 1], f32, tag="lnv")
            nc.vector.tensor_scalar(lnv[:], lnc[:], -1.0, None, op0=Alu.mult)
            # v = mag - 0.5*u ; t = exp(v - ln(colsum)) = exp(mag)/(mag*colsum)
            v = sbig.tile([P, N], f32, tag="v")
            nc.vector.scalar_tensor_tensor(v[:], u[:], -0.5, mag[:],
                                           op0=Alu.mult, op1=Alu.add)
            t = sbig.tile([P, N], f32, tag="t")
            nc.scalar.activation(t[:], v[:], Act.Exp, bias=lnv[:, 0:1])
            if c == 0:
                ddump("sr0", sr[:])
                ddump("t0", t[:])
            ddump("cs", colsum[:], c)
            # modulation weights (bf16)
            ar = sbf.tile([P, N], bf16, tag="ar")
            nc.vector.tensor_mul(ar[:], t[:], sr[:])
            ai = sbf.tile([P, N], bf16, tag="ai")
            nc.vector.tensor_mul(ai[:], t[:], si[:])
            asum = sbf.tile([P, N], bf16, tag="asum")
            nc.vector.tensor_add(asum[:], ar[:], ai[:])
            if c == 0:
                ddump("ar0", ar[:])

            # --- products + PE reductions ---
            for k in range(K1):
                p1 = prp.tile([P, N], bf16, tag="p1")
                nc.vector.tensor_mul(p1[:], lr_t[k][:], ar[:])
                p2 = prp.tile([P, N], bf16, tag="p2")
                nc.vector.tensor_mul(p2[:], li_t[k][:], ai[:])
                p3 = prp.tile([P, N], bf16, tag="p3")
                nc.vector.tensor_mul(p3[:], ls_t[k][:], asum[:])
                if c == 0 and k == 0:
                    ddump("p10", p1[:])
                first = (c == 0 and k == 0)
                last = (c == CH - 1 and k == K1 - 1)
                for h in range(2):
                    hs = slice(h * 512, (h + 1) * 512)
                    nc.tensor.matmul(slp[:, hs], pairw[k][0][:], p1[:, hs],
                                     start=first, stop=False,
                                     skip_group_check=True)
                    nc.tensor.matmul(slp[:, hs], pairw[k][1][:], p2[:, hs],
                                     start=False, stop=False,
                                     skip_group_check=True)
                    nc.tensor.matmul(slp[:, hs], pairw[k][2][:], p3[:, hs],
                                     start=False, stop=last,
                                     skip_group_check=True)

    # ---------------- einsum tail ----------------
    with tc.tile_pool(name="eins", bufs=1) as ep, \
         tc.tile_pool(name="bc_pool", bufs=2) as bcp, \
         tc.tile_pool(name="v_pool", bufs=2) as vp, \
         tc.tile_pool(name="out_psum", bufs=1, space="PSUM") as op:
        sl_bf = ep.tile([2 * K1, N], bf16)
        nc.scalar.copy(sl_bf[:], slp[:])
        ddump("slbf", sl_bf[:])
        nc.sync.dma_start(sl_dram[:], sl_bf[:])

        outr_ps = op.tile([OC, N], f32)
        outi_ps = op.tile([OC, N], f32)
        for k in range(K1):
            slr = sl_dram[2 * k:2 * k + 1, :].partition_broadcast(C)
            sli = sl_dram[2 * k + 1:2 * k + 2, :].partition_broadcast(C)
            b12 = bcp.tile([P, N], bf16, tag="b12")
            nc.sync.dma_start(b12[0:C, :], slr)
            nc.scalar.dma_start(b12[C:2 * C, :], sli)
            b34 = bcp.tile([P, N], bf16, tag="b34")
            nc.sync.dma_start(b34[0:C, :], sli)
            nc.scalar.dma_start(b34[C:2 * C, :], slr)
            v12 = vp.tile([P, N], bf16, tag="v12")
            nc.vector.tensor_mul(v12[:], xt_bf[:], b12[:])
            v34 = vp.tile([P, N], bf16, tag="v34")
            nc.vector.tensor_mul(v34[:], xt_bf[:], b34[:])
            if k == 0:
                ddump("b120", b12[:])
                ddump("v120", v12[:])
            ks = slice(k * OC, (k + 1) * OC)
            for h in range(2):
                hs = slice(h * 512, (h + 1) * 512)
                nc.tensor.matmul(outr_ps[:, hs], wcr_sb[:, ks], v12[:, hs],
                                 start=(k == 0), stop=(k == K1 - 1),
                                 skip_group_check=True)
                nc.tensor.matmul(outi_ps[:, hs], wci_sb[:, ks], v34[:, hs],
                                 start=(k == 0), stop=(k == K1 - 1),
                                 skip_group_check=True)

        outr_sb = ep.tile([OC, N], f32)
        nc.scalar.copy(outr_sb[:], outr_ps[:])
        outi_sb = ep.tile([OC, N], f32)
        nc.scalar.copy(outi_sb[:], outi_ps[:])
        nc.sync.dma_start(out_rt[:], outr_sb[:])
        nc.sync.dma_start(out_it[:], outi_sb[:])
    sl_psum_ctx.close()


def _host_prep(inputs):
    """Build per-core input maps from full inputs."""
    import ml_dtypes
    bf16 = ml_dtypes.bfloat16
    f = lambda k: np.asarray(inputs[k], dtype=np.float32)
    X_real, X_imag = f("X_real"), f("X_imag")
    L_real, L_imag = f("L_real"), f("L_imag")
    w_real, w_imag = f("w_real"), f("w_imag")
    aw_real, aw_imag = f("aw_real"), f("aw_imag")
    ab_real = float(np.asarray(inputs["ab_real"]))
    ab_imag = float(np.asarray(inputs["ab_imag"]))
    pa_real = float(np.asarray(inputs["pa_real"]))
    pa_imag = float(np.asarray(inputs["pa_imag"]))

    ws_r, wd_r = aw_real[:C], aw_real[C:]
    ws_i, wd_i = aw_imag[:C], aw_imag[C:]
    w4 = np.stack([
        np.concatenate([ws_r, -ws_i]),
        np.concatenate([ws_i, ws_r]),
        np.concatenate([wd_r, -wd_i]),
        np.concatenate([wd_i, wd_r]),
    ], axis=1).astype(np.float32)                      # [128, 4]

    # wcat_r = [w_r[k]; -w_i[k]] over c2, free=(k,o); wcat_i = [w_r[k]; w_i[k]]
    wr_t = w_real.transpose(1, 0, 2).reshape(C, K1 * OC)
    wi_t = w_imag.transpose(1, 0, 2).reshape(C, K1 * OC)
    wcat_r = np.concatenate([wr_t, -wi_t], axis=0).astype(bf16)
    wcat_i = np.concatenate([wr_t, wi_t], axis=0).astype(bf16)

    pa_cols = np.stack([np.full(P, pa_real), np.full(P, pa_imag)],
                       axis=1).astype(np.float32)       # [128, 2]
    abr = np.array([[ab_real]], dtype=np.float32)
    abi = np.array([[ab_imag]], dtype=np.float32)

    # transposed L streams, chunked [CH, K1, 128, N], bf16
    # lrt[c,k,jj,i] = L_real[k, i, c*128+jj]
    def t_chunks(L):
        # [K1, N(i), N(j)] -> [K1, N(j), N(i)] -> [CH, K1, P, N]
        Lt = np.ascontiguousarray(L.transpose(0, 2, 1))
        return np.ascontiguousarray(
            Lt.reshape(K1, CH, P, N).transpose(1, 0, 2, 3)).astype(bf16)

    in_maps = []
    for b in range(NCORES):
        Ls = L_real[b] + L_imag[b]
        in_maps.append({
            "x_real": np.ascontiguousarray(X_real[b]),
            "x_imag": np.ascontiguousarray(X_imag[b]),
            "lrt": t_chunks(L_real[b]),
            "lit": t_chunks(L_imag[b]),
            "lst": t_chunks(Ls),
            "w4": w4, "wcat_r": wcat_r, "wcat_i": wcat_i,
            "pa_cols": pa_cols, "abr": abr, "abi": abi,
        })
    return in_maps


def kernel(**inputs):
    import os
    from concourse import bass_utils

    if "nc" not in _CACHE:
        _CACHE["nc"] = _build_nc()
    nc = _CACHE["nc"]
    in_maps = _host_prep(inputs)
    trace = os.environ.get("KERNEL_TRACE", "0") == "1"
    res = bass_utils.run_bass_kernel_spmd(nc, in_maps,
                                          core_ids=list(range(NCORES)),
                                          trace=trace)
    _CACHE["last_result"] = res
    out_r = np.stack([res.results[b]["out_rt"].T for b in range(NCORES)])
    out_i = np.stack([res.results[b]["out_it"].T for b in range(NCORES)])
    return np.ascontiguousarray(out_r), np.ascontiguousarray(out_i)
